# revision 16
# baseline (speedup 1.0000x reference)
# Trainium2 Bass kernel for relative-position causal attention
# (Transformer-XL style: logits = q·k + q·table[n-m], causal softmax, AV, out-proj).
#
# Sharding: tensor-parallel over heads — 16 heads / 8 cores = 2 heads per core.
# Each core computes its heads' projections, attention, and a partial output
# projection [B,D,N] fp16; the host sums the 8 partials in fp32.
#
# Position logits are computed in *diagonal* layout (T[ni, j] = q[nb+ni]·table[j],
# a plain matmul since the table index is the diagonal n-m), then converted to
# row layout with a DMA "shear" through a DRAM scratch strip: partition ni reads
# flat offset ni*(P-1) + c with row pitch P = W+128, which is exactly the
# per-partition-shifted gather no on-chip engine can do. The 128-column pad of
# each strip row is pre-poisoned with -1e30, so the causal mask falls out of the
# pitch arithmetic for free (row ni's reads beyond column n land in the pad).
#
# Precision: every logit-affecting matmul runs as a 3-term bf16 hi/lo split
# (a·b ≈ ah·bh + ah·bl + al·bh, error ~2^-16 relative) instead of true fp32 —
# same accuracy class for this problem (top-2 logit gaps ~0.014 at |logit|~8e4
# need absolute error << 1), but 3 PE cycles/row instead of 4, and bf16 enables
# 2-head row-tiled PE packing (both heads' K=64 matmuls run concurrently in the
# 128x128 array via tile_position). The value path (v, softmax weights, AV,
# out-proj) runs in fp16. P and V transposes go through the DMA XBAR
# (dma_start_transpose) instead of the PE.

from contextlib import ExitStack

import numpy as np

N = 2048
M = 2048
B = 2
D = 1024
H = 16
DQK = 64
DV = 64
NCORES = 8
HPC = H // NCORES  # heads per core = 2
NZ = N * B
KT = D // 128  # 8 contraction tiles
TW = 2048  # table width (diagonals 0..2047)
NEG = -1.0e30

_cache = {}


def _split_bf16(a):
    """hi = bf16(a), lo = bf16(a - hi), as uint16-viewed bfloat16 arrays."""
    import ml_dtypes

    a = np.asarray(a, dtype=np.float32)
    hi = a.astype(ml_dtypes.bfloat16)
    lo = (a - hi.astype(np.float32)).astype(ml_dtypes.bfloat16)
    return np.ascontiguousarray(hi), np.ascontiguousarray(lo)


def _sincos_rev():
    """sincos basis for diagonals d=0..2047, column-reversed, transposed to
    [D, TW] so sctr[:, jr] = sincos(d=TW-1-jr).  Computed with jax on CPU to
    match the reference's fp32 rounding of inv_freq/phases/sin bitwise."""
    try:
        import jax
        import jax.numpy as jnp

        cpu = jax.devices("cpu")[0]
        with jax.default_device(cpu):
            r = jnp.arange(0.0, float(TW), dtype=jnp.float32)
            inv_freq = 1.0 / (
                10000.0 ** (jnp.arange(0.0, D, 2.0, dtype=jnp.float32) / D)
            )
            phases = r[:, None] * inv_freq[None, :]
            sincos = jnp.concatenate([jnp.sin(phases), jnp.cos(phases)], axis=-1)
            sc = np.asarray(sincos)  # [TW, D]
    except Exception:
        r = np.arange(0.0, float(TW), dtype=np.float32)
        inv_freq = (
            1.0
            / (10000.0 ** (np.arange(0.0, D, 2.0, dtype=np.float32) / np.float32(D)))
        ).astype(np.float32)
        phases = (r[:, None] * inv_freq[None, :]).astype(np.float32)
        sc = np.concatenate(
            [np.sin(phases, dtype=np.float32), np.cos(phases, dtype=np.float32)],
            axis=-1,
        )
    return np.ascontiguousarray(sc[::-1].T.astype(np.float32))  # [D, TW]


USE_DMA_T = True   # DMA XBAR transposes for P and V (else PE transposes)
USE_TTR = False    # fused TTR crashes the deployed runtime; use add+reduce_max
SKIP_ATTN = False  # debug: bypass phase_attn (attn16 <- memset)
SKIP_AV = False    # debug: run attn up to ptT, skip AV matmuls + attn copy
SKIP_SHEAR = False # debug: skip strip write/read; Lp <- memset
STOP_AT = 9        # debug: 1=diag only, 2=+content/TTR, 3=+exp/mul, 4=+transpose, 9=full
SEQ_HEADS = False  # debug: serialize heads, baseline-style per-head AV psum


def _build(loop=1, dbg=False):
    import concourse.bacc as bacc
    import concourse.mybir as mybir
    import concourse.tile as tile
    from concourse.bass import AP
    from concourse.tile_rust import add_dep_helper

    f32 = mybir.dt.float32
    f16 = mybir.dt.float16
    bf16 = mybir.dt.bfloat16
    AX = mybir.AxisListType.X
    MAX = mybir.AluOpType.max
    ADD = mybir.AluOpType.add
    SUB = mybir.AluOpType.subtract
    EXP = mybir.ActivationFunctionType.Exp

    nc = bacc.Bacc("TRN2", target_bir_lowering=False, debug=False, num_devices=NCORES)

    xqh = nc.dram_tensor("xqh", [D, NZ], bf16, kind="ExternalInput")
    xql = nc.dram_tensor("xql", [D, NZ], bf16, kind="ExternalInput")
    xkh = nc.dram_tensor("xkh", [D, NZ], bf16, kind="ExternalInput")
    xkl = nc.dram_tensor("xkl", [D, NZ], bf16, kind="ExternalInput")
    sch = nc.dram_tensor("sch", [D, TW], bf16, kind="ExternalInput")
    scl = nc.dram_tensor("scl", [D, TW], bf16, kind="ExternalInput")
    wqh = nc.dram_tensor("wqh", [D, 128], bf16, kind="ExternalInput")
    wql = nc.dram_tensor("wql", [D, 128], bf16, kind="ExternalInput")
    wkh = nc.dram_tensor("wkh", [D, 128], bf16, kind="ExternalInput")
    wkl = nc.dram_tensor("wkl", [D, 128], bf16, kind="ExternalInput")
    wph = nc.dram_tensor("wph", [D, 128], bf16, kind="ExternalInput")
    wpl = nc.dram_tensor("wpl", [D, 128], bf16, kind="ExternalInput")
    wv = nc.dram_tensor("wv", [D, 128], bf16, kind="ExternalInput")
    wo = nc.dram_tensor("wo", [128, D], f16, kind="ExternalInput")
    outT = nc.dram_tensor("outT", [B, D, N], f16, kind="ExternalOutput")
    dbg_t = {}
    if dbg:
        for nm, shp, dt in [
            ("d_qh", [128, NZ], bf16),
            ("d_ql", [128, NZ], bf16),
            ("d_kh", [128, NZ], bf16),
            ("d_th", [128, TW], bf16),
            ("d_V", [128, B * 16 * 128], f16),
            ("d_L15", [128, 2048], f32),
            ("d_P15", [128, 2048], f16),
            ("d_pt15", [128, 16 * 128], f16),
            ("d_attn16", [128, NZ], f16),
        ]:
            dbg_t[nm] = nc.dram_tensor(nm, shp, dt, kind="ExternalOutput")

    # DRAM scratch strips for the diagonal->row shear, one per (z, h, i).
    # Row pitch P = W + 128; the last 128 columns of each row are the poison
    # pad (written once, outside the timing loop) that realizes the causal
    # mask during the sheared read.
    scr = {}
    for z in range(B):
        for h in range(HPC):
            for i in range(16):
                W = 128 * (i + 1)
                P = W + 128
                scr[(z, h, i)] = nc.dram_tensor(
                    f"scr_{z}_{h}_{i}", [128 * P + 128], f32, kind="Internal"
                )

    with tile.TileContext(nc) as tc:
        with ExitStack() as ctx:
            wpool = ctx.enter_context(tc.tile_pool(name="wpool", bufs=1))
            big = ctx.enter_context(tc.tile_pool(name="big", bufs=1))
            work = ctx.enter_context(tc.tile_pool(name="work", bufs=3))

            # ---- weights to SBUF ----
            w_sb = {}
            for nm, dram, dt in [
                ("qh", wqh, bf16),
                ("ql", wql, bf16),
                ("kh", wkh, bf16),
                ("kl", wkl, bf16),
                ("ph", wph, bf16),
                ("pl", wpl, bf16),
                ("v", wv, bf16),
            ]:
                t = wpool.tile([128, KT, 128], dt, tag=f"w{nm}", name=f"w{nm}")
                nc.sync.dma_start(t[:], dram.ap().rearrange("(t p) m -> p t m", p=128))
                w_sb[nm] = t
            wo_sb = wpool.tile([128, D], f16)
            nc.sync.dma_start(wo_sb[:], wo.ap())

            poison = wpool.tile([128, 128], f32)
            nc.vector.memset(poison[:], NEG)
            ident16 = wpool.tile([128, 128], f16)
            if not USE_DMA_T:
                from concourse.masks import make_identity

                make_identity(nc, ident16[:])

            # ---- poison pads: once per launch, outside the timing loop ----
            pad_writes = {}
            for z in range(B):
                for h in range(HPC):
                    for i in range(16):
                        W = 128 * (i + 1)
                        P = W + 128
                        pad_writes[(z, h, i)] = nc.sync.dma_start(
                            AP(scr[(z, h, i)], W, [[P, 128], [1, 128]]), poison[:]
                        )

            # ---- persistent activations ----
            q_hi = big.tile([128, NZ], bf16)  # [hd, z*N+n]
            q_lo = big.tile([128, NZ], bf16)
            k_hi = big.tile([128, NZ], bf16)
            k_lo = big.tile([128, NZ], bf16)
            t_hi = big.tile([128, TW], bf16)  # reversed diagonal table
            t_lo = big.tile([128, TW], bf16)
            V_all = big.tile([128, B * 16, 128], f16)  # [m-part, z*16+mb, hv]
            attn16 = big.tile([128, NZ], f16)  # [hv, z*N+n], normalized

            def split_out(pp_t, hi_sb, lo_sb, sl):
                """psum fp32 chunk -> hi (bf16, ACT) and lo = x - hi (bf16, DVE)."""
                nc.scalar.copy(hi_sb[:, sl], pp_t[:])
                nc.vector.tensor_tensor(
                    out=lo_sb[:, sl], in0=pp_t[:], in1=hi_sb[:, sl], op=SUB
                )

            def phase_proj():
                with tc.tile_pool(name="pp", bufs=1, space="PSUM") as pp, tc.tile_pool(
                    name="xp", bufs=2
                ) as xp:
                    # --- q projection: 2 half-passes of 2048 cols, 4 psum chunks ---
                    for half in range(2):
                        cs = 2048 * half
                        psq = [
                            pp.tile([128, 512], f32, tag=f"pj{c}", name=f"q{half}{c}")
                            for c in range(4)
                        ]
                        for t in range(KT):
                            xh_t = xp.tile([128, 2048], bf16, tag="xh")
                            xl_t = xp.tile([128, 2048], bf16, tag="xl")
                            nc.sync.dma_start(
                                xh_t[:],
                                xqh.ap().rearrange("(t p) n -> p t n", p=128)[
                                    :, t, cs : cs + 2048
                                ],
                            )
                            nc.sync.dma_start(
                                xl_t[:],
                                xql.ap().rearrange("(t p) n -> p t n", p=128)[
                                    :, t, cs : cs + 2048
                                ],
                            )
                            for c in range(4):
                                rh = xh_t[:, 512 * c : 512 * (c + 1)]
                                rl = xl_t[:, 512 * c : 512 * (c + 1)]
                                nc.tensor.matmul(
                                    psq[c][:], w_sb["qh"][:, t, :], rh,
                                    start=(t == 0), stop=False,
                                )
                                nc.tensor.matmul(
                                    psq[c][:], w_sb["qh"][:, t, :], rl,
                                    start=False, stop=False,
                                )
                                nc.tensor.matmul(
                                    psq[c][:], w_sb["ql"][:, t, :], rh,
                                    start=False, stop=(t == KT - 1),
                                )
                        for c in range(4):
                            split_out(
                                psq[c], q_hi, q_lo,
                                slice(cs + 512 * c, cs + 512 * (c + 1)),
                            )

                    # --- k+v projection: shared x tiles, 4+4 psum chunks ---
                    vT16 = xp.tile([128, NZ], f16, tag="vT")
                    for half in range(2):
                        cs = 2048 * half
                        psk = [
                            pp.tile([128, 512], f32, tag=f"pj{c}", name=f"k{half}{c}")
                            for c in range(4)
                        ]
                        psv = [
                            pp.tile([128, 512], f32, tag=f"pv{c}", name=f"v{half}{c}")
                            for c in range(4)
                        ]
                        for t in range(KT):
                            xh_t = xp.tile([128, 2048], bf16, tag="xh")
                            xl_t = xp.tile([128, 2048], bf16, tag="xl")
                            nc.sync.dma_start(
                                xh_t[:],
                                xkh.ap().rearrange("(t p) n -> p t n", p=128)[
                                    :, t, cs : cs + 2048
                                ],
                            )
                            nc.sync.dma_start(
                                xl_t[:],
                                xkl.ap().rearrange("(t p) n -> p t n", p=128)[
                                    :, t, cs : cs + 2048
                                ],
                            )
                            for c in range(4):
                                rh = xh_t[:, 512 * c : 512 * (c + 1)]
                                rl = xl_t[:, 512 * c : 512 * (c + 1)]
                                nc.tensor.matmul(
                                    psk[c][:], w_sb["kh"][:, t, :], rh,
                                    start=(t == 0), stop=False,
                                )
                                nc.tensor.matmul(
                                    psk[c][:], w_sb["kh"][:, t, :], rl,
                                    start=False, stop=False,
                                )
                                nc.tensor.matmul(
                                    psk[c][:], w_sb["kl"][:, t, :], rh,
                                    start=False, stop=(t == KT - 1),
                                )
                                nc.tensor.matmul(
                                    psv[c][:], w_sb["v"][:, t, :], rh,
                                    start=(t == 0), stop=(t == KT - 1),
                                )
                        for c in range(4):
                            split_out(
                                psk[c], k_hi, k_lo,
                                slice(cs + 512 * c, cs + 512 * (c + 1)),
                            )
                            nc.scalar.copy(
                                vT16[:, cs + 512 * c : cs + 512 * (c + 1)], psv[c][:]
                            )

                    # --- table projection: one pass of 2048 cols ---
                    pst = [
                        pp.tile([128, 512], f32, tag=f"pj{c}", name=f"t{c}")
                        for c in range(4)
                    ]
                    for t in range(KT):
                        xh_t = xp.tile([128, 2048], bf16, tag="xh")
                        xl_t = xp.tile([128, 2048], bf16, tag="xl")
                        nc.sync.dma_start(
                            xh_t[:], sch.ap().rearrange("(t p) n -> p t n", p=128)[:, t, :]
                        )
                        nc.sync.dma_start(
                            xl_t[:], scl.ap().rearrange("(t p) n -> p t n", p=128)[:, t, :]
                        )
                        for c in range(4):
                            rh = xh_t[:, 512 * c : 512 * (c + 1)]
                            rl = xl_t[:, 512 * c : 512 * (c + 1)]
                            nc.tensor.matmul(
                                pst[c][:], w_sb["ph"][:, t, :], rh,
                                start=(t == 0), stop=False,
                            )
                            nc.tensor.matmul(
                                pst[c][:], w_sb["ph"][:, t, :], rl,
                                start=False, stop=False,
                            )
                            nc.tensor.matmul(
                                pst[c][:], w_sb["pl"][:, t, :], rh,
                                start=False, stop=(t == KT - 1),
                            )
                    for c in range(4):
                        split_out(pst[c], t_hi, t_lo, slice(512 * c, 512 * (c + 1)))

                    # --- V tiles [m, hv] ---
                    if USE_DMA_T:
                        nc.sync.dma_start_transpose(
                            V_all[:].rearrange("p a b -> p (a b)").rearrange(
                                "p (a b) -> p a b", b=128
                            ),
                            vT16[:],
                        )
                    else:
                        for zb in range(B * 16):
                            vp = pp.tile([128, 128], f16, tag="pj0", name="vp")
                            nc.tensor.transpose(
                                vp[:],
                                vT16[:, 128 * zb : 128 * (zb + 1)],
                                ident16[:],
                            )
                            nc.vector.tensor_copy(V_all[:, zb, :], vp[:])

            def phase_attn():
                with tc.tile_pool(name="pa1", bufs=1, space="PSUM") as pa1, tc.tile_pool(
                    name="pa2", bufs=2, space="PSUM"
                ) as pa2, tc.tile_pool(name="tpo", bufs=2) as tpo, tc.tile_pool(
                    name="lpo", bufs=2
                ) as lpo, tc.tile_pool(name="ppo", bufs=2) as ppo:
                    if SKIP_AV or STOP_AT < 9:
                        nc.vector.memset(attn16[:], 0.001)
                    for z in range(B):
                        for i in range(16):
                            nb = 128 * i
                            W = nb + 128
                            P = W + 128
                            nchunks = (W + 511) // 512  # 512-col psum chunks
                            lastw = W - 512 * (nchunks - 1)
                            q_b = [
                                (q_hi[64 * h : 64 * (h + 1), z * N + nb : z * N + W],
                                 q_lo[64 * h : 64 * (h + 1), z * N + nb : z * N + W])
                                for h in range(2)
                            ]

                            # --- position logits in reversed diagonal layout ---
                            tT = [
                                tpo.tile([128, 2048], f32, tag=f"tT{h}", name=f"tT{h}")
                                for h in range(2)
                            ]
                            for bc in range(nchunks):
                                wdt = min(512, W - 512 * bc)
                                tps = [
                                    pa1.tile([128, 512], f32, tag=f"tps{h}", name=f"tps{h}")
                                    for h in range(2)
                                ]
                                for h in range(2):
                                    hs = slice(64 * h, 64 * (h + 1))
                                    th_c = t_hi[hs, TW - W + 512 * bc : TW - W + 512 * bc + wdt]
                                    tl_c = t_lo[hs, TW - W + 512 * bc : TW - W + 512 * bc + wdt]
                                    qh_b, ql_b = q_b[h]
                                    nc.tensor.matmul(
                                        tps[h][:, :wdt], qh_b, th_c,
                                        start=True, stop=False,
                                    )
                                    nc.tensor.matmul(
                                        tps[h][:, :wdt], qh_b, tl_c,
                                        start=False, stop=False,
                                    )
                                    nc.tensor.matmul(
                                        tps[h][:, :wdt], ql_b, th_c,
                                        start=False, stop=True,
                                    )
                                for h in range(2):
                                    # alternate ACT/DVE for the psum->sbuf copies
                                    if (bc + h) % 2 == 0:
                                        nc.scalar.copy(
                                            tT[h][:, 512 * bc : 512 * bc + wdt],
                                            tps[h][:, :wdt],
                                        )
                                    else:
                                        nc.vector.tensor_copy(
                                            tT[h][:, 512 * bc : 512 * bc + wdt],
                                            tps[h][:, :wdt],
                                        )
                            w_inst = {}
                            if not SKIP_SHEAR:
                                for h in range(2):
                                    s = scr[(z, h, i)]
                                    w_inst[h] = nc.sync.dma_start(
                                        AP(s, 0, [[P, 128], [1, W]]), tT[h][:, :W]
                                    )

                            if STOP_AT < 2:
                                sink = work.tile([128, 1], f32, tag="sink")
                                nc.vector.reduce_max(out=sink[:], in_=tT[0][:, :4], axis=AX)
                                nc.vector.reduce_max(out=sink[:], in_=tT[1][:, :4], axis=AX)
                                continue
                            # --- sheared read: row-layout position logits ---
                            Lp = [
                                lpo.tile([128, 2048], f32, tag=f"Lp{h}", name=f"Lp{h}")
                                for h in range(2)
                            ]
                            for h in range(2):
                                if SKIP_SHEAR:
                                    nc.vector.memset(Lp[h][:, :W], -1.0)
                                    continue
                                s = scr[(z, h, i)]
                                r_inst = nc.sync.dma_start(
                                    Lp[h][:, :W],
                                    AP(s, 127, [[P - 1, 128], [1, W]]),
                                )
                                add_dep_helper(
                                    r_inst.ins, w_inst[h].ins,
                                    reason="shear read after strip write",
                                )
                                add_dep_helper(
                                    r_inst.ins, pad_writes[(z, h, i)].ins,
                                    reason="shear read after poison pad",
                                )

                            # --- content logits + add + rowmax (fused TTR) ---
                            L_sb = [
                                lpo.tile([128, 2048], f32, tag=f"L{h}", name=f"L{h}")
                                for h in range(2)
                            ]
                            cmax = work.tile([128, 2, 4], f32, tag="cmax")
                            for c in range(nchunks):
                                wdt = min(512, W - 512 * c)
                                cps = [
                                    pa2.tile([128, 512], f32, tag=f"cps{h}", name=f"cps{h}")
                                    for h in range(2)
                                ]
                                for h in range(2):
                                    hs = slice(64 * h, 64 * (h + 1))
                                    kh_c = k_hi[hs, z * N + 512 * c : z * N + 512 * c + wdt]
                                    kl_c = k_lo[hs, z * N + 512 * c : z * N + 512 * c + wdt]
                                    qh_b, ql_b = q_b[h]
                                    nc.tensor.matmul(
                                        cps[h][:, :wdt], qh_b, kh_c,
                                        start=True, stop=False,
                                    )
                                    nc.tensor.matmul(
                                        cps[h][:, :wdt], qh_b, kl_c,
                                        start=False, stop=False,
                                    )
                                    nc.tensor.matmul(
                                        cps[h][:, :wdt], ql_b, kh_c,
                                        start=False, stop=True,
                                    )
                                for h in range(2):
                                    if USE_TTR:
                                        nc.vector.tensor_tensor_reduce(
                                            out=L_sb[h][:, 512 * c : 512 * c + wdt],
                                            in0=cps[h][:, :wdt],
                                            in1=Lp[h][:, 512 * c : 512 * c + wdt],
                                            scale=1.0,
                                            scalar=NEG,
                                            op0=ADD,
                                            op1=MAX,
                                            accum_out=cmax[:, h, c : c + 1],
                                        )
                                    else:
                                        nc.vector.tensor_add(
                                            L_sb[h][:, 512 * c : 512 * c + wdt],
                                            cps[h][:, :wdt],
                                            Lp[h][:, 512 * c : 512 * c + wdt],
                                        )
                                        nc.vector.reduce_max(
                                            out=cmax[:, h, c : c + 1],
                                            in_=L_sb[h][:, 512 * c : 512 * c + wdt],
                                            axis=AX,
                                        )

                            if STOP_AT < 3:
                                continue
                            # --- softmax: exp with safe max, sums, normalize ---
                            ptT = [
                                ppo.tile([128, 16, 128], f16, tag=f"pt{h}", name=f"pt{h}")
                                for h in range(2)
                            ]
                            for h in range(2):
                                negmax = work.tile([128, 1], f32, tag=f"ngm{h}")
                                nc.vector.tensor_reduce(
                                    out=negmax[:],
                                    in_=cmax[:, h, :nchunks],
                                    axis=AX,
                                    op=MAX,
                                    negate=True,
                                )
                                P_sb = ppo.tile([128, 2048], f16, tag=f"P{h}")
                                ssum = work.tile([128, 1], f32, tag=f"ssum{h}")
                                nc.scalar.activation(
                                    P_sb[:, :W],
                                    L_sb[h][:, :W],
                                    EXP,
                                    bias=negmax[:],
                                    scale=1.0,
                                    accum_out=ssum[:],
                                )
                                rsum = work.tile([128, 1], f32, tag=f"rsum{h}")
                                nc.vector.reciprocal(rsum[:], ssum[:])
                                nc.vector.tensor_scalar_mul(
                                    P_sb[:, :W], P_sb[:, :W], rsum[:]
                                )
                                if STOP_AT < 4:
                                    nc.vector.tensor_copy(
                                        ptT[h][:, 0, :], P_sb[:, :128]
                                    )
                                elif USE_DMA_T:
                                    nc.sync.dma_start_transpose(
                                        ptT[h][:, : i + 1, :], P_sb[:, :W]
                                    )
                                else:
                                    for g in range((i + 4) // 4):
                                        ng = min(4, i + 1 - 4 * g)
                                        ptp = pa1.tile(
                                            [128, 512], f16, tag="ptp", name="ptp"
                                        )
                                        for u in range(ng):
                                            nc.tensor.transpose(
                                                ptp[:, 128 * u : 128 * (u + 1)],
                                                P_sb[:, 128 * (4 * g + u) : 128 * (4 * g + u + 1)],
                                                ident16[:],
                                            )
                                        nc.vector.tensor_copy(
                                            ptT[h][:, 4 * g : 4 * g + ng, :],
                                            ptp[:, : 128 * ng],
                                        )
                                if dbg and z == 0 and h == 0 and i == 15:
                                    nc.sync.dma_start(dbg_t["d_L15"].ap(), L_sb[h][:])
                                    nc.sync.dma_start(dbg_t["d_P15"].ap(), P_sb[:])
                                    nc.sync.dma_start(
                                        dbg_t["d_pt15"].ap(),
                                        ptT[h][:].rearrange("p a b -> p (a b)"),
                                    )

                            # --- AV: 2-head column-packed accumulation ---
                            if SKIP_AV:
                                nc.vector.tensor_copy(
                                    attn16[:, z * N + nb : z * N + nb + 128],
                                    ptT[0][:, 0, :].rearrange("p a b -> p (a b)")
                                    if False
                                    else ptT[0][:, 0, :],
                                )
                                nc.vector.tensor_copy(
                                    attn16[:, z * N + nb : z * N + nb + 128],
                                    ptT[1][:, 0, :],
                                )
                                continue
                            at2 = pa1.tile([128, 128], f32, tag="at2")
                            for mt in range(i + 1):
                                for h in range(2):
                                    hs = slice(64 * h, 64 * (h + 1))
                                    nc.tensor.matmul(
                                        at2[hs, :],
                                        V_all[:, z * 16 + mt, hs],
                                        ptT[h][:, mt, :],
                                        start=(mt == 0),
                                        stop=(mt == i),
                                        skip_group_check=True,
                                    )
                            nc.vector.tensor_copy(
                                attn16[:, z * N + nb : z * N + nb + 128], at2[:]
                            )

            def phase_out(pa):
                if dbg:
                    nc.sync.dma_start(dbg_t["d_qh"].ap(), q_hi[:])
                    nc.sync.dma_start(dbg_t["d_ql"].ap(), q_lo[:])
                    nc.sync.dma_start(dbg_t["d_kh"].ap(), k_hi[:])
                    nc.sync.dma_start(dbg_t["d_th"].ap(), t_hi[:])
                    nc.sync.dma_start(
                        dbg_t["d_V"].ap(), V_all[:].rearrange("p a b -> p (a b)")
                    )
                    nc.sync.dma_start(dbg_t["d_attn16"].ap(), attn16[:])
                for z in range(B):
                    for dc in range(8):
                        for nn in range(4):
                            o_ps = pa.tile([128, 512], f32, tag="ops")
                            nc.tensor.matmul(
                                o_ps[:],
                                wo_sb[:, 128 * dc : 128 * (dc + 1)],
                                attn16[:, z * N + 512 * nn : z * N + 512 * (nn + 1)],
                                start=True,
                                stop=True,
                            )
                            o_sb = work.tile([128, 512], f16, tag="osb")
                            nc.scalar.copy(o_sb[:], o_ps[:])
                            nc.sync.dma_start(
                                outT.ap()[
                                    z,
                                    128 * dc : 128 * (dc + 1),
                                    512 * nn : 512 * (nn + 1),
                                ],
                                o_sb[:],
                            )

            def loop_body():
                phase_proj()
                if SKIP_ATTN:
                    nc.vector.memset(attn16[:], 0.001)
                else:
                    phase_attn()
                with tc.tile_pool(name="po", bufs=2, space="PSUM") as po:
                    phase_out(po)

            if loop == 1:
                loop_body()
            else:
                with tc.For_i(0, loop, 1):
                    loop_body()

    nc.compile()
    return nc


def _prep_inputs(x_q, x_kv, to_q, to_kv, for_pos_enc, to_o):
    xq = np.asarray(x_q, dtype=np.float32).transpose(2, 1, 0).reshape(D, NZ)
    xkv = np.asarray(x_kv, dtype=np.float32).transpose(2, 1, 0).reshape(D, NZ)
    xqh, xql = _split_bf16(xq)
    xkh, xkl = _split_bf16(xkv)
    if "sctr" not in _cache:
        sctr = _sincos_rev()
        _cache["sctr"] = _split_bf16(sctr)
    sch, scl = _cache["sctr"]
    to_q = np.asarray(to_q, dtype=np.float32)
    to_kv = np.asarray(to_kv, dtype=np.float32)
    fpe = np.asarray(for_pos_enc, dtype=np.float32)
    to_o = np.asarray(to_o, dtype=np.float32)
    in_maps = []
    for c in range(NCORES):
        hs = slice(HPC * c, HPC * (c + 1))
        wq = np.ascontiguousarray(to_q[hs].reshape(HPC * DQK, D).T)
        wk = np.ascontiguousarray(to_kv[hs, :DQK].reshape(HPC * DQK, D).T)
        wvc = np.ascontiguousarray(to_kv[hs, DQK:].reshape(HPC * DV, D).T)
        wp = np.ascontiguousarray(fpe[hs].reshape(HPC * DQK, D).T)
        woc = np.ascontiguousarray(to_o[:, hs, :].reshape(D, HPC * DV).T).astype(
            np.float16
        )
        wq_h, wq_l = _split_bf16(wq)
        wk_h, wk_l = _split_bf16(wk)
        wp_h, wp_l = _split_bf16(wp)
        wv_h, _ = _split_bf16(wvc)
        in_maps.append(
            {
                "xqh": xqh, "xql": xql, "xkh": xkh, "xkl": xkl,
                "sch": sch, "scl": scl,
                "wqh": wq_h, "wql": wq_l,
                "wkh": wk_h, "wkl": wk_l,
                "wph": wp_h, "wpl": wp_l,
                "wv": wv_h, "wo": woc,
            }
        )
    return in_maps


def kernel(x_q, x_kv, to_q, to_kv, for_pos_enc, to_o):
    from concourse.bass_utils import run_bass_kernel_spmd

    if "nc" not in _cache:
        _cache["nc"] = _build()
    nc = _cache["nc"]
    in_maps = _prep_inputs(x_q, x_kv, to_q, to_kv, for_pos_enc, to_o)
    res = run_bass_kernel_spmd(nc, in_maps, core_ids=list(range(NCORES)))
    acc = np.zeros((B, D, N), dtype=np.float32)
    for c in range(NCORES):
        acc += res.results[c]["outT"].astype(np.float32)
    return np.ascontiguousarray(acc.transpose(2, 0, 1)).astype(np.float32)
